# revision 17
# baseline (speedup 1.0000x reference)
"""Trainium2 Bass kernel for nn_Colorizer (retrieval_knn).

Computation (per reference frame r of 3, per pixel p of a 128x128 image):
  corr[r, n, p] = <feats_t[:, p], feats_r[r, :, p + offset(n)]>   n in 13x13
  q_val[r, p]  = max_n corr ; q_idx[r, p] = argmax_n corr
  gathered[r, c, p] = quantized_sub[r, c, p + offset(q_idx)]      (zero pad)
  out[c, p] = sum_r softmax_r(q_val)[r] * gathered[r, c, p]

Sharding: spatial h dim split into 8 bands of 16 rows (one per core); each
core handles all 3 refs for its band so the ref softmax is local.

Device pipeline per (ref, tile) of 128 pixels (16x8) x 560-window (28x20):
  PE    : corr Gram via fp16 hi/lo split (x256 prescale) - 6 matmuls/tile,
          ~2e-7 relative accuracy, 3 cycles/col vs fp32's 4.
  DVE   : ONE fused custom op (masked argmax): in0=corr psum, in1=t-table
          (valid pos -> 1024*k + n_patch, invalid -> distinct big negatives),
          m=in0+min(in1,0); fold select(eq(m, runmax), in1) with MAX accum
          = encoded (window k, patch n') of the argmax.
  ACT   : psum -> fp16 copy (q/2^20) for the later q_val fetch.
  GPSIMD: batched indirect_copy "overgather" (16x group-shared lists) of the
          argmax patch pixel (u8x4-packed) and q_val, then masked diagonal
          reduction extracts each partition's own value.
  DVE   : tiny index arithmetic per 12-tile chunk + final softmax combine.
"""

import os

import numpy as np

import concourse.bass as bass
import concourse.mybir as mybir
import concourse.tile as tile
from concourse import bacc
from concourse.bass_utils import run_bass_kernel_spmd

F32 = mybir.dt.float32
F16 = mybir.dt.float16
BF16 = mybir.dt.bfloat16
U8 = mybir.dt.uint8
U16 = mybir.dt.uint16
U32 = mybir.dt.uint32
ALU = mybir.AluOpType

NCORES = 8
NREF, C, H, W = 3, 128, 128, 128
RAD = 6                      # patch radius
PS = 2 * RAD + 1             # 13
NP = PS * PS                 # 169 patch positions
CQ = 3                       # quantized channels
SUB = 4                      # quantized_r spatial subsample stride

ROWS = H // NCORES           # 16 rows per core
XB = 8                       # x block size
NT = W // XB                 # 16 tiles per ref
WY = ROWS + 2 * RAD          # 28 window rows
WX = XB + 2 * RAD            # 20 window cols
WIN = WY * WX                # 560
HALF = WY // 2               # 14 window rows per PSUM bank
NHALF = HALF * WX            # 280 columns per matmul
PW = W + 2 * RAD             # 140 padded width
NRT = NREF * NT              # 48 (ref, tile) pairs
NCH = 12                     # tiles per gather chunk
NCHK = NRT // NCH            # 4 chunks
GRP = 16                     # indirect_copy partition group size

SCALE = 256.0                # input prescale (fp16 split exactness)
QSC = 1.0 / (65536.0 * 16.0)  # psum -> fp16 qval copy scale
QEXP = 16.0                  # exp() scale undoing the /16

_CACHE: dict = {}


def _register_argmax_op():
    """Fused masked argmax DVE op (runtime-registered, table per NEFF)."""
    from concourse import dve_ops
    from concourse.dve_spec import (
        Spec, Src0, Src1, Zero, MaxNeg, minn, eq, select, scan, AluOp, lower,
    )
    from concourse.dve_uop import DveOpSpec

    name = "MASKED_ARGMAX_ANT"
    for o in dve_ops.OPS:
        if o.name == name:
            return o

    m = Src0 + minn(Src1, Zero)
    r = scan(AluOp.MAX, m)
    body = select(eq(m, r), Src1, MaxNeg)

    def ref(in0, in1, s0, s1, imm2):
        x = in0.astype(np.float32) + np.minimum(in1.astype(np.float32), 0.0)
        rr = np.maximum.accumulate(x, axis=-1)
        sel = np.where(x == rr, in1, -np.finfo(np.float32).max)
        return sel, sel.max(axis=-1, keepdims=True)

    spec = Spec(body=body, accum=AluOp.MAX, reference=ref)
    row = dve_ops._CUSTOM_DVE_ROW_BASE + len(dve_ops.OPS)
    shas = {}
    for ver in ("v3", "v4"):
        s = DveOpSpec(name=name, opcode=row, uops=lower(spec, ver=ver),
                      rd1_en=True)
        shas[ver] = s.sha(ver)
    op = dve_ops.DveOp(name, spec, subdim=False, uops_sha=shas)
    dve_ops.OPS.append(op)
    dve_ops._SUB_OPCODE_FOR_NAME[name] = row
    dve_ops.CUSTOM_DVE_SPECS[name] = spec
    return op


def _build_program() -> bacc.Bacc:
    amx = _register_argmax_op()
    nc = bacc.Bacc("TRN2", target_bir_lowering=False, debug=False)

    fth_d = nc.dram_tensor("fth", [C, ROWS * W], F16, kind="ExternalInput")
    ftl_d = nc.dram_tensor("ftl", [C, ROWS * W], F16, kind="ExternalInput")
    frh_d = nc.dram_tensor("frh", [NREF, C, WY * PW], F16, kind="ExternalInput")
    frl_d = nc.dram_tensor("frl", [NREF, C, WY * PW], F16, kind="ExternalInput")
    tt_d = nc.dram_tensor("tt", [128, WIN], F32, kind="ExternalInput")
    qrp_d = nc.dram_tensor("qrp", [NCHK, 128, NCH * NP], U32,
                           kind="ExternalInput")
    crv_d = nc.dram_tensor("crv", [128, NCH], F32, kind="ExternalInput")
    crq_d = nc.dram_tensor("crq", [128, NCH], F32, kind="ExternalInput")
    d32_d = nc.dram_tensor("d32", [128, GRP], U32, kind="ExternalInput")
    d16_d = nc.dram_tensor("d16", [128, GRP], F16, kind="ExternalInput")
    out_d = nc.dram_tensor("out", [128, NT * 4], F32, kind="ExternalOutput")

    with tile.TileContext(nc) as tc:
        with (
            tc.tile_pool(name="const", bufs=1) as constp,
            tc.tile_pool(name="qrpp", bufs=2) as qrpp,
            tc.tile_pool(name="psum", bufs=4, space="PSUM") as psump,
            tc.tile_pool(name="stream", bufs=2) as streamp,
            tc.tile_pool(name="small", bufs=1) as smallp,
        ):
            # ---- input DMAs, staged so tile (r0,t0) starts early ----
            fth_sb = constp.tile([C, ROWS * W], F16, tag="fth")
            nc.sync.dma_start(out=fth_sb[:, 0:512], in_=fth_d.ap()[:, 0:512])
            frh_sb, frl_sb = [], []
            for r in range(NREF):
                th_ = constp.tile([C, WY * PW], F16, tag=f"frh{r}")
                frh_sb.append(th_)
                tl_ = constp.tile([C, WY * PW], F16, tag=f"frl{r}")
                frl_sb.append(tl_)
            fh0 = frh_sb[0][:].rearrange("c (y x) -> c y x", x=PW)
            fh0d = frh_d.ap()[0].rearrange("c (y x) -> c y x", x=PW)
            fl0 = frl_sb[0][:].rearrange("c (y x) -> c y x", x=PW)
            fl0d = frl_d.ap()[0].rearrange("c (y x) -> c y x", x=PW)
            tt_sb = constp.tile([128, WIN], F32, tag="tt")
            ftl_sb = constp.tile([C, ROWS * W], F16, tag="ftl")
            nc.sync.dma_start(out=fh0[:, :, 0:44], in_=fh0d[:, :, 0:44])
            nc.sync.dma_start(out=tt_sb[:], in_=tt_d.ap())
            nc.sync.dma_start(out=ftl_sb[:, 0:512], in_=ftl_d.ap()[:, 0:512])
            nc.sync.dma_start(out=fl0[:, :, 0:44], in_=fl0d[:, :, 0:44])
            nc.sync.dma_start(out=fth_sb[:, 512:], in_=fth_d.ap()[:, 512:])
            nc.sync.dma_start(out=ftl_sb[:, 512:], in_=ftl_d.ap()[:, 512:])
            nc.sync.dma_start(out=fh0[:, :, 44:76], in_=fh0d[:, :, 44:76])
            nc.sync.dma_start(out=fl0[:, :, 44:76], in_=fl0d[:, :, 44:76])
            nc.sync.dma_start(out=fh0[:, :, 76:PW], in_=fh0d[:, :, 76:PW])
            nc.sync.dma_start(out=fl0[:, :, 76:PW], in_=fl0d[:, :, 76:PW])
            crv_sb = constp.tile([128, NCH], F32, tag="crv")
            nc.sync.dma_start(out=crv_sb[:], in_=crv_d.ap())
            crq_sb = constp.tile([128, NCH], F32, tag="crq")
            nc.sync.dma_start(out=crq_sb[:], in_=crq_d.ap())
            d32_sb = constp.tile([128, GRP], U32, tag="d32")
            nc.sync.dma_start(out=d32_sb[:], in_=d32_d.ap())
            d16_sb = constp.tile([128, GRP], F16, tag="d16")
            nc.sync.dma_start(out=d16_sb[:], in_=d16_d.ap())
            for r in range(1, NREF):
                nc.sync.dma_start(out=frh_sb[r][:], in_=frh_d.ap()[r])
                nc.sync.dma_start(out=frl_sb[r][:], in_=frl_d.ap()[r])
            qrp_sb = []
            for ck in range(NCHK):
                t_ = qrpp.tile([128, NCH * NP], U32, tag="qrp")
                nc.sync.dma_start(out=t_[:], in_=qrp_d.ap()[ck])
                qrp_sb.append(t_)

            vfold, qc16 = [], []
            for ck in range(NCHK):
                vf_ = smallp.tile([128, NCH], F32, tag=f"vfold{ck}")
                vfold.append(vf_)
                qc_ = smallp.tile([128, NCH * WIN], F16, tag=f"qc16{ck}")
                qc16.append(qc_)
            qvals = smallp.tile([128, NRT], F32, tag="qvals")
            vsel = smallp.tile([128, NRT], U32, tag="vsel")

            # pre-warm the Exp activation table (hides ACT_TABLE_LOAD)
            warm = smallp.tile([128, 1], F32, tag="warm")
            nc.gpsimd.memset(warm[:], 0.0)
            nc.scalar.activation(out=warm[:], in_=warm[:],
                                 func=mybir.ActivationFunctionType.Exp)

            ttv = tt_sb[:].rearrange("p (b n) -> p b n", b=2)

            ogv_t, ogq_t = [None] * NCHK, [None] * NCHK

            def chunk_gather(ck):
                """Index arithmetic (DVE-only deps) + overgathers (gpsimd)
                for tiles [12ck, 12ck+12)."""
                vf = vfold[ck][:]
                kf = smallp.tile([128, NCH], F32, tag=f"kf{ck}")
                nc.vector.tensor_scalar_mul(out=kf[:], in0=vf, scalar1=1.0 / 1024.0)
                ku = smallp.tile([128, NCH], U16, tag=f"ku{ck}")
                nc.vector.tensor_copy(out=ku[:], in_=kf[:])
                kfi = smallp.tile([128, NCH], F32, tag=f"kfi{ck}")
                nc.vector.tensor_copy(out=kfi[:], in_=ku[:])
                k1024 = smallp.tile([128, NCH], F32, tag=f"k1k{ck}")
                nc.vector.tensor_scalar_mul(out=k1024[:], in0=kfi[:], scalar1=1024.0)
                npf = smallp.tile([128, NCH], F32, tag=f"npf{ck}")
                nc.vector.tensor_tensor(out=npf[:], in0=vf, in1=k1024[:],
                                        op=ALU.subtract)
                vidxf = smallp.tile([128, NCH], F32, tag=f"vif{ck}")
                nc.vector.tensor_tensor(out=vidxf[:], in0=npf[:], in1=crv_sb[:],
                                        op=ALU.add)
                vidx = smallp.tile([128, NCH], U16, tag=f"vi{ck}")
                nc.vector.tensor_copy(out=vidx[:], in_=vidxf[:])
                qidxf = smallp.tile([128, NCH], F32, tag=f"qif{ck}")
                nc.vector.tensor_tensor(out=qidxf[:], in0=kfi[:],
                                        in1=crq_sb[:], op=ALU.add)
                qidx = smallp.tile([128, NCH], U16, tag=f"qi{ck}")
                nc.vector.tensor_copy(out=qidx[:], in_=qidxf[:])

                ogv = smallp.tile([128, NCH * GRP], U32, tag=f"ogv{ck}")
                nc.gpsimd.indirect_copy(out=ogv[:], data=qrp_sb[ck][:],
                                        idxs=vidx[:],
                                        i_know_ap_gather_is_preferred=True)
                ogq = smallp.tile([128, NCH * GRP], F16, tag=f"ogq{ck}")
                nc.gpsimd.indirect_copy(out=ogq[:], data=qc16[ck][:],
                                        idxs=qidx[:],
                                        i_know_ap_gather_is_preferred=True)
                # diag pre-mask on the (otherwise idle) gpsimd queue
                d16b = d16_sb[:].rearrange("p (o m) -> p o m", o=1) \
                    .to_broadcast([128, NCH, GRP])
                qmul = smallp.tile([128, NCH * GRP], F16, tag=f"qm{ck}")
                nc.gpsimd.tensor_tensor(
                    out=qmul[:].rearrange("p (t m) -> p t m", m=GRP),
                    in0=ogq[:].rearrange("p (t m) -> p t m", m=GRP),
                    in1=d16b, op=ALU.mult)
                ogv_t[ck], ogq_t[ck] = ogv, qmul

            def chunk_reduce(ck):
                """Diagonal extraction (DVE) - emitted well after the gathers
                so the DVE queue head never waits on gpsimd."""
                lo, hi = ck * NCH, (ck + 1) * NCH
                ogv, qmul = ogv_t[ck], ogq_t[ck]
                d32b = d32_sb[:].rearrange("p (o m) -> p o m", o=1) \
                    .to_broadcast([128, NCH, GRP])
                vand = smallp.tile([128, NCH * GRP], U32, tag=f"va{ck}")
                nc.vector.tensor_tensor(
                    out=vand[:].rearrange("p (t m) -> p t m", m=GRP),
                    in0=ogv[:].rearrange("p (t m) -> p t m", m=GRP),
                    in1=d32b, op=ALU.bitwise_and)
                nc.vector.tensor_reduce(
                    out=vsel[:, lo:hi],
                    in_=vand[:].rearrange("p (t m) -> p t m", m=GRP),
                    axis=mybir.AxisListType.X, op=ALU.bitwise_or)
                nc.vector.tensor_reduce(
                    out=qvals[:, lo:hi],
                    in_=qmul[:].rearrange("p (t m) -> p t m", m=GRP),
                    axis=mybir.AxisListType.X, op=ALU.add)

            # ---- main loop: r outer so ref 0 overlaps refs 1/2 input DMA ----
            gather_at = {13: 0, 25: 1, 37: 2}
            reduce_at = {20: 0, 32: 1, 44: 2}
            for r in range(NREF):
                fhv = frh_sb[r][:].rearrange("c (y x) -> c y x", x=PW)
                flv = frl_sb[r][:].rearrange("c (y x) -> c y x", x=PW)
                for t in range(NT):
                    rt = r * NT + t
                    ps = psump.tile([128, 1024], F32, tag="ps")
                    lh = fth_sb[:, t * 128:(t + 1) * 128]
                    ll = ftl_sb[:, t * 128:(t + 1) * 128]
                    x0 = t * XB
                    rh1 = fhv[:, 0:HALF, x0:x0 + WX]
                    rh2 = fhv[:, HALF:WY, x0:x0 + WX]
                    rl1 = flv[:, 0:HALF, x0:x0 + WX]
                    rl2 = flv[:, HALF:WY, x0:x0 + WX]
                    nc.tensor.matmul(ps[:, 0:NHALF], lh, rh1,
                                     start=True, stop=False)
                    nc.tensor.matmul(ps[:, 512:512 + NHALF], lh, rh2,
                                     start=True, stop=False)
                    nc.tensor.matmul(ps[:, 0:NHALF], lh, rl1,
                                     start=False, stop=False)
                    nc.tensor.matmul(ps[:, 512:512 + NHALF], lh, rl2,
                                     start=False, stop=False)
                    nc.tensor.matmul(ps[:, 0:NHALF], ll, rh1,
                                     start=False, stop=True)
                    nc.tensor.matmul(ps[:, 512:512 + NHALF], ll, rh2,
                                     start=False, stop=True)

                    psv = ps[:].rearrange("p (b n) -> p b n", b=2)[:, :, 0:NHALF]
                    sc = streamp.tile([128, WIN], F16, tag="sc")
                    nc.vector._custom_dve(
                        amx,
                        out=sc[:].rearrange("p (b n) -> p b n", b=2),
                        in0=psv, in1=ttv,
                        accum_out=vfold[rt // NCH][:, rt % NCH:rt % NCH + 1],
                    )
                    qcv = qc16[rt // NCH][:, (rt % NCH) * WIN:(rt % NCH + 1) * WIN] \
                        .rearrange("p (b n) -> p b n", b=2)
                    nc.scalar.activation(
                        out=qcv, in_=psv,
                        func=mybir.ActivationFunctionType.Copy, scale=QSC)

                    if rt in gather_at:
                        chunk_gather(gather_at[rt])
                    if rt in reduce_at:
                        chunk_reduce(reduce_at[rt])
            chunk_gather(NCHK - 1)
            chunk_reduce(NCHK - 1)

            # ---- unpack gathered u8x4 values -> f32 ----
            valsf = smallp.tile([128, NRT * 4], F32, tag="valsf")
            nc.vector.tensor_copy(out=valsf[:], in_=vsel[:].bitcast(U8))

            # ---- softmax over refs + weighted sum ----
            qvv = qvals[:].rearrange("p (r t) -> p r t", r=NREF)
            m01 = smallp.tile([128, NT], F32, tag="m01")
            nc.vector.tensor_tensor(out=m01[:], in0=qvv[:, 0], in1=qvv[:, 1],
                                    op=ALU.max)
            mmx = smallp.tile([128, NT], F32, tag="mmx")
            nc.vector.tensor_tensor(out=mmx[:], in0=m01[:], in1=qvv[:, 2],
                                    op=ALU.max)
            es = []
            for r in range(NREF):
                e_ = smallp.tile([128, NT], F32, tag=f"e{r}")
                nc.vector.tensor_tensor(out=e_[:], in0=qvv[:, r], in1=mmx[:],
                                        op=ALU.subtract)
                nc.scalar.activation(out=e_[:], in_=e_[:],
                                     func=mybir.ActivationFunctionType.Exp,
                                     scale=QEXP)
                es.append(e_)
            ssum = smallp.tile([128, NT], F32, tag="ssum")
            nc.vector.tensor_tensor(out=ssum[:], in0=es[0][:], in1=es[1][:],
                                    op=ALU.add)
            nc.vector.tensor_tensor(out=ssum[:], in0=ssum[:], in1=es[2][:],
                                    op=ALU.add)
            rec = smallp.tile([128, NT], F32, tag="rec")
            nc.vector.reciprocal(out=rec[:], in_=ssum[:])
            nc.vector.tensor_scalar_mul(out=rec[:], in0=rec[:],
                                        scalar1=1.0 / 255.0)

            gvv = valsf[:].rearrange("p (r t c) -> p r t c", r=NREF, c=4)
            oacc = smallp.tile([128, NT * 4], F32, tag="oacc")
            oaccv = oacc[:].rearrange("p (t c) -> p t c", c=4)
            for r in range(NREF):
                w_ = smallp.tile([128, NT], F32, tag=f"w{r}")
                nc.vector.tensor_tensor(out=w_[:], in0=es[r][:], in1=rec[:],
                                        op=ALU.mult)
                wb = w_[:].rearrange("p (t o) -> p t o", o=1) \
                    .to_broadcast([128, NT, 4])
                if r == 0:
                    nc.vector.tensor_tensor(out=oaccv, in0=gvv[:, r], in1=wb,
                                            op=ALU.mult)
                else:
                    term = smallp.tile([128, NT * 4], F32, tag=f"term{r}")
                    termv = term[:].rearrange("p (t c) -> p t c", c=4)
                    nc.vector.tensor_tensor(out=termv, in0=gvv[:, r], in1=wb,
                                            op=ALU.mult)
                    nc.vector.tensor_tensor(out=oaccv, in0=oaccv, in1=termv,
                                            op=ALU.add)
            nc.sync.dma_start(out=out_d.ap(), in_=oacc[:])


    nc.compile()
    return nc


def _host_prep(feats_r, feats_t, quantized_r):
    """Build the 8 per-core input maps."""
    from numpy.lib.stride_tricks import sliding_window_view

    # prescaled, zero-padded refs; fp16 hi/lo split
    frp = np.zeros((NREF, C, H + 2 * RAD, PW), np.float32)
    frp[:, :, RAD:RAD + H, RAD:RAD + W] = feats_r[:, 0] * SCALE
    frh_full = frp.astype(np.float16)
    frl_full = (frp - frh_full.astype(np.float32)).astype(np.float16)

    ft = feats_t[0] * SCALE                      # [C, H, W] f32
    fth_full = ft.astype(np.float16)
    ftl_full = (ft - fth_full.astype(np.float32)).astype(np.float16)

    # t-table: valid -> 1024*k + n', invalid -> distinct decreasing negatives
    yl = np.arange(ROWS)[:, None, None, None]
    xl = np.arange(XB)[None, :, None, None]
    wy = np.arange(WY)[None, None, :, None]
    wx = np.arange(WX)[None, None, None, :]
    dy, dx = wy - yl, wx - xl
    valid = (dy >= 0) & (dy < PS) & (dx >= 0) & (dx < PS)
    kk = (wy * WX + wx).astype(np.float32)
    npr = (dy * PS + dx).astype(np.float32)
    tt = np.where(valid, 1024.0 * kk + npr,
                  (-1.0e30 * (1.0 + kk / 1024.0)).astype(np.float32))
    tt = np.ascontiguousarray(
        (tt + np.zeros((ROWS, XB, WY, WX), np.float32)).reshape(128, WIN),
        np.float32)

    # quantized refs: subsample, u8-encode, pad, per-pixel 13x13 patch tables
    qr = np.asarray(quantized_r[:, 0, :, ::SUB, ::SUB], np.float32)
    qr_u8 = np.clip(np.round(qr * 255.0), 0, 255).astype(np.uint8)
    qpad = np.zeros((NREF, CQ, H + 2 * RAD, W + 2 * RAD), np.uint8)
    qpad[:, :, RAD:RAD + H, RAD:RAD + W] = qr_u8
    sw = sliding_window_view(qpad, (PS, PS), axis=(2, 3))  # [r,c,H,W,13,13]

    crv = np.broadcast_to(
        (np.arange(NCH, dtype=np.float32) * NP)[None, :], (128, NCH))
    crv = np.ascontiguousarray(crv)
    crq = np.broadcast_to(
        (np.arange(NCH, dtype=np.float32) * WIN)[None, :], (128, NCH))
    crq = np.ascontiguousarray(crq)
    pm = np.arange(128) % GRP
    d32 = (pm[:, None] == np.arange(GRP)[None, :]).astype(np.uint32) * \
        np.uint32(0xFFFFFFFF)
    d16 = (pm[:, None] == np.arange(GRP)[None, :]).astype(np.float16)

    in_maps = []
    for k in range(NCORES):
        y0 = ROWS * k
        ft_h = np.ascontiguousarray(
            fth_full[:, y0:y0 + ROWS, :].reshape(C, ROWS, NT, XB)
            .transpose(0, 2, 1, 3).reshape(C, ROWS * W))
        ft_l = np.ascontiguousarray(
            ftl_full[:, y0:y0 + ROWS, :].reshape(C, ROWS, NT, XB)
            .transpose(0, 2, 1, 3).reshape(C, ROWS * W))
        fr_h = np.ascontiguousarray(
            frh_full[:, :, y0:y0 + WY, :].reshape(NREF, C, WY * PW))
        fr_l = np.ascontiguousarray(
            frl_full[:, :, y0:y0 + WY, :].reshape(NREF, C, WY * PW))

        # patch tables [p=(yl,xl), (r,t), n'=(dy,dx)] as u8x4 packed in u32
        blk = sw[:, :, y0:y0 + ROWS, :, :, :]          # [r, c, 16, 128, 13, 13]
        b2 = blk.reshape(NREF, CQ, ROWS, NT, XB, PS, PS)
        b3 = b2.transpose(2, 4, 0, 3, 5, 6, 1)         # [yl, xl, r, t, dy, dx, c]
        tbl = np.zeros((ROWS, XB, NREF, NT, NP, 4), np.uint8)
        tbl[..., :CQ] = np.ascontiguousarray(b3).reshape(
            ROWS, XB, NREF, NT, NP, CQ)
        qrp = tbl.reshape(128, NRT * NP * 4).view(np.uint32) \
            .reshape(128, NRT * NP)
        qrp = np.ascontiguousarray(
            qrp.reshape(128, NCHK, NCH * NP).transpose(1, 0, 2))

        in_maps.append({
            "fth": ft_h, "ftl": ft_l, "frh": fr_h, "frl": fr_l,
            "tt": tt, "qrp": qrp, "crv": crv, "crq": crq,
            "d32": d32, "d16": d16,
        })
    return in_maps


def _install_ntff_shim():
    """This container's antenv lacks axon_hooks, so run_bass_kernel_spmd's
    trace path can't find the NTFF profile hook. Inject the module and
    register the ctypes-based hook from the boot script. Best-effort."""
    try:
        import sys
        import types

        if "antenv.axon_hooks" in sys.modules:
            return
        mod = types.ModuleType("antenv.axon_hooks")
        holder = [None]
        mod.set_axon_ntff_profile_hook = lambda h: holder.__setitem__(0, h)
        mod.get_axon_ntff_profile_hook = lambda: holder[0]
        sys.modules["antenv.axon_hooks"] = mod
        import antenv

        antenv.axon_hooks = mod
        from trn_agent_boot.trn_boot import _ntff_profile_via_ctypes

        hook = _ntff_profile_via_ctypes("/opt/axon/libaxon_pjrt.so")
        if hook is not None:
            mod.set_axon_ntff_profile_hook(hook)
    except Exception as e:  # pragma: no cover - tracing is best-effort
        print(f"ntff shim install failed: {e}")


last_exec_time_ns = None
last_results = None


def kernel(feats_r, feats_t, quantized_r, ref_index=None, current_ind=None):
    global last_exec_time_ns, last_results
    feats_r = np.asarray(feats_r, np.float32)
    feats_t = np.asarray(feats_t, np.float32)
    quantized_r = np.asarray(quantized_r, np.float32)

    in_maps = _host_prep(feats_r, feats_t, quantized_r)

    if "nc" not in _CACHE:
        _CACHE["nc"] = _build_program()
    nc = _CACHE["nc"]

    trace = bool(int(os.environ.get("KERNEL_TRACE", "0")))
    kwargs = {}
    if trace:
        _install_ntff_shim()
        tdir = os.environ.get("KERNEL_TRACE_DIR")
        if tdir:
            os.makedirs(tdir, exist_ok=True)
            kwargs["tmpdir"] = tdir
    res = run_bass_kernel_spmd(
        nc, in_maps, list(range(NCORES)), trace=trace, **kwargs
    )
    last_exec_time_ns = res.exec_time_ns
    last_results = res.results

    out = np.concatenate(
        [_unshard_core(res.results[k]["out"]) for k in range(NCORES)], axis=1
    )
    return np.ascontiguousarray(out.reshape(1, CQ, H, W), np.float32)


def _unshard_core(raw):
    # raw [128, NT*4]: partition p=(yl,xl), free (t, c4) -> [CQ, ROWS, W]
    r = np.asarray(raw).reshape(ROWS, XB, NT, 4)[..., :CQ]
    return r.transpose(3, 0, 2, 1).reshape(CQ, ROWS, W)


# revision 19
# speedup vs baseline: 1.0654x; 1.0654x over previous
"""Trainium2 Bass kernel for nn_Colorizer (retrieval_knn).

Computation (per reference frame r of 3, per pixel p of a 128x128 image):
  corr[r, n, p] = <feats_t[:, p], feats_r[r, :, p + offset(n)]>   n in 13x13
  q_val[r, p]  = max_n corr ; q_idx[r, p] = argmax_n corr
  gathered[r, c, p] = quantized_sub[r, c, p + offset(q_idx)]      (zero pad)
  out[c, p] = sum_r softmax_r(q_val)[r] * gathered[r, c, p]

Sharding: spatial h dim split into 8 bands of 16 rows (one per core); each
core handles all 3 refs for its band so the ref softmax is local.

Device pipeline per (ref, tile) of 128 pixels (16x8) x 560-window (28x20):
  PE    : corr Gram via fp16 hi/lo split (x256 prescale) - 6 matmuls/tile,
          ~2e-7 relative accuracy, 3 cycles/col vs fp32's 4.
  DVE   : ONE fused custom op (masked argmax): in0=corr psum, in1=t-table
          (valid pos -> 1024*k + n_patch, invalid -> distinct big negatives),
          m=in0+min(in1,0); fold select(eq(m, runmax), in1) with MAX accum
          = encoded (window k, patch n') of the argmax.
  ACT   : psum -> fp16 copy (q/2^20) for the later q_val fetch.
  GPSIMD: batched indirect_copy "overgather" (16x group-shared lists) of the
          argmax patch pixel (u8x4-packed) and q_val, then masked diagonal
          reduction extracts each partition's own value.
  DVE   : tiny index arithmetic per 12-tile chunk + final softmax combine.
"""

import os

import numpy as np

import concourse.bass as bass
import concourse.mybir as mybir
import concourse.tile as tile
from concourse import bacc
from concourse.bass_utils import run_bass_kernel_spmd

F32 = mybir.dt.float32
F16 = mybir.dt.float16
BF16 = mybir.dt.bfloat16
U8 = mybir.dt.uint8
U16 = mybir.dt.uint16
U32 = mybir.dt.uint32
ALU = mybir.AluOpType

NCORES = 8
NREF, C, H, W = 3, 128, 128, 128
RAD = 6                      # patch radius
PS = 2 * RAD + 1             # 13
NP = PS * PS                 # 169 patch positions
CQ = 3                       # quantized channels
SUB = 4                      # quantized_r spatial subsample stride

ROWS = H // NCORES           # 16 rows per core
XB = 8                       # x block size
NT = W // XB                 # 16 tiles per ref
WY = ROWS + 2 * RAD          # 28 window rows
WX = XB + 2 * RAD            # 20 window cols
WIN = WY * WX                # 560
HALF = WY // 2               # 14 window rows per PSUM bank
NHALF = HALF * WX            # 280 columns per matmul
PW = W + 2 * RAD             # 140 padded width
NRT = NREF * NT              # 48 (ref, tile) pairs
NCH = 12                     # tiles per gather chunk
NCHK = NRT // NCH            # 4 chunks
GRP = 16                     # indirect_copy partition group size

SCALE = 256.0                # input prescale (fp16 split exactness)
QSC = 1.0 / (65536.0 * 16.0)  # psum -> fp16 qval copy scale
QEXP = 16.0                  # exp() scale undoing the /16

_CACHE: dict = {}


def _register_argmax_op():
    """Fused masked argmax DVE op (runtime-registered, table per NEFF)."""
    from concourse import dve_ops
    from concourse.dve_spec import (
        Spec, Src0, Src1, Zero, MaxNeg, minn, eq, select, scan, AluOp, lower,
    )
    from concourse.dve_uop import DveOpSpec

    name = "MASKED_ARGMAX_ANT"
    for o in dve_ops.OPS:
        if o.name == name:
            return o

    m = Src0 + minn(Src1, Zero)
    r = scan(AluOp.MAX, m)
    body = select(eq(m, r), Src1, MaxNeg)

    def ref(in0, in1, s0, s1, imm2):
        x = in0.astype(np.float32) + np.minimum(in1.astype(np.float32), 0.0)
        rr = np.maximum.accumulate(x, axis=-1)
        sel = np.where(x == rr, in1, -np.finfo(np.float32).max)
        return sel, sel.max(axis=-1, keepdims=True)

    spec = Spec(body=body, accum=AluOp.MAX, reference=ref)
    row = dve_ops._CUSTOM_DVE_ROW_BASE + len(dve_ops.OPS)
    shas = {}
    for ver in ("v3", "v4"):
        s = DveOpSpec(name=name, opcode=row, uops=lower(spec, ver=ver),
                      rd1_en=True)
        shas[ver] = s.sha(ver)
    op = dve_ops.DveOp(name, spec, subdim=False, uops_sha=shas)
    dve_ops.OPS.append(op)
    dve_ops._SUB_OPCODE_FOR_NAME[name] = row
    dve_ops.CUSTOM_DVE_SPECS[name] = spec
    return op


def _build_program() -> bacc.Bacc:
    amx = _register_argmax_op()
    nc = bacc.Bacc("TRN2", target_bir_lowering=False, debug=False)

    fth_d = nc.dram_tensor("fth", [C, ROWS * W], F16, kind="ExternalInput")
    ftl_d = nc.dram_tensor("ftl", [C, ROWS * W], F16, kind="ExternalInput")
    frh_d = nc.dram_tensor("frh", [NREF, C, WY * PW], F16, kind="ExternalInput")
    frl_d = nc.dram_tensor("frl", [NREF, C, WY * PW], F16, kind="ExternalInput")
    tt_d = nc.dram_tensor("tt", [128, WIN], F32, kind="ExternalInput")
    qrp_d = nc.dram_tensor("qrp", [NCHK, 128, NCH * NP], U32,
                           kind="ExternalInput")
    crv_d = nc.dram_tensor("crv", [128, NCH], F32, kind="ExternalInput")
    crq_d = nc.dram_tensor("crq", [128, NCH], F32, kind="ExternalInput")
    d32_d = nc.dram_tensor("d32", [128, GRP], U32, kind="ExternalInput")
    d16_d = nc.dram_tensor("d16", [128, GRP], F16, kind="ExternalInput")
    out_d = nc.dram_tensor("out", [128, NT * 4], F32, kind="ExternalOutput")

    with tile.TileContext(nc) as tc:
        with (
            tc.tile_pool(name="const", bufs=1) as constp,
            tc.tile_pool(name="qrpp", bufs=2) as qrpp,
            tc.tile_pool(name="psum", bufs=4, space="PSUM") as psump,
            tc.tile_pool(name="stream", bufs=2) as streamp,
            tc.tile_pool(name="small", bufs=1) as smallp,
        ):
            # ---- input DMAs, staged so tile (r0,t0) starts early ----
            fth_sb = constp.tile([C, ROWS * W], F16, tag="fth")
            nc.sync.dma_start(out=fth_sb[:, 0:512], in_=fth_d.ap()[:, 0:512])
            frh_sb, frl_sb = [], []
            for r in range(NREF):
                th_ = constp.tile([C, WY * PW], F16, tag=f"frh{r}")
                frh_sb.append(th_)
                tl_ = constp.tile([C, WY * PW], F16, tag=f"frl{r}")
                frl_sb.append(tl_)
            fh0 = frh_sb[0][:].rearrange("c (y x) -> c y x", x=PW)
            fh0d = frh_d.ap()[0].rearrange("c (y x) -> c y x", x=PW)
            fl0 = frl_sb[0][:].rearrange("c (y x) -> c y x", x=PW)
            fl0d = frl_d.ap()[0].rearrange("c (y x) -> c y x", x=PW)
            tt_sb = constp.tile([128, WIN], F32, tag="tt")
            ftl_sb = constp.tile([C, ROWS * W], F16, tag="ftl")
            nc.sync.dma_start(out=fh0[:, :, 0:44], in_=fh0d[:, :, 0:44])
            nc.sync.dma_start(out=tt_sb[:], in_=tt_d.ap())
            nc.sync.dma_start(out=ftl_sb[:, 0:512], in_=ftl_d.ap()[:, 0:512])
            nc.sync.dma_start(out=fl0[:, :, 0:44], in_=fl0d[:, :, 0:44])
            nc.sync.dma_start(out=fth_sb[:, 512:], in_=fth_d.ap()[:, 512:])
            nc.sync.dma_start(out=ftl_sb[:, 512:], in_=ftl_d.ap()[:, 512:])
            nc.sync.dma_start(out=fh0[:, :, 44:76], in_=fh0d[:, :, 44:76])
            nc.sync.dma_start(out=fl0[:, :, 44:76], in_=fl0d[:, :, 44:76])
            nc.sync.dma_start(out=fh0[:, :, 76:PW], in_=fh0d[:, :, 76:PW])
            nc.sync.dma_start(out=fl0[:, :, 76:PW], in_=fl0d[:, :, 76:PW])
            crv_sb = constp.tile([128, NCH], F32, tag="crv")
            nc.sync.dma_start(out=crv_sb[:], in_=crv_d.ap())
            crq_sb = constp.tile([128, NCH], F32, tag="crq")
            nc.sync.dma_start(out=crq_sb[:], in_=crq_d.ap())
            d32_sb = constp.tile([128, GRP], U32, tag="d32")
            nc.sync.dma_start(out=d32_sb[:], in_=d32_d.ap())
            d16_sb = constp.tile([128, GRP], F16, tag="d16")
            nc.sync.dma_start(out=d16_sb[:], in_=d16_d.ap())
            for r in range(1, NREF):
                nc.sync.dma_start(out=frh_sb[r][:], in_=frh_d.ap()[r])
                nc.sync.dma_start(out=frl_sb[r][:], in_=frl_d.ap()[r])
            qrp_sb = []
            for ck in range(NCHK):
                t_ = qrpp.tile([128, NCH * NP], U32, tag="qrp")
                nc.sync.dma_start(out=t_[:], in_=qrp_d.ap()[ck])
                qrp_sb.append(t_)

            vfold, qc16 = [], []
            for ck in range(NCHK):
                vf_ = smallp.tile([128, NCH], F32, tag=f"vfold{ck}")
                vfold.append(vf_)
                qc_ = smallp.tile([128, NCH * WIN], F16, tag=f"qc16{ck}")
                qc16.append(qc_)
            qvals = smallp.tile([128, NRT], F32, tag="qvals")
            vsel = smallp.tile([128, NRT], U32, tag="vsel")

            # pre-warm the Exp activation table (hides ACT_TABLE_LOAD)
            warm = smallp.tile([128, 1], F32, tag="warm")
            nc.gpsimd.memset(warm[:], 0.0)
            nc.scalar.activation(out=warm[:], in_=warm[:],
                                 func=mybir.ActivationFunctionType.Exp)

            ttv = tt_sb[:].rearrange("p (b n) -> p b n", b=2)

            ogv_t, ogq_t = [None] * NCHK, [None] * NCHK

            def chunk_gather(ck):
                """Index arithmetic (DVE-only deps) + overgathers (gpsimd)
                for tiles [12ck, 12ck+12)."""
                vf = vfold[ck][:]
                kf = smallp.tile([128, NCH], F32, tag=f"kf{ck}")
                nc.vector.tensor_scalar_mul(out=kf[:], in0=vf, scalar1=1.0 / 1024.0)
                ku = smallp.tile([128, NCH], U16, tag=f"ku{ck}")
                nc.vector.tensor_copy(out=ku[:], in_=kf[:])
                kfi = smallp.tile([128, NCH], F32, tag=f"kfi{ck}")
                nc.vector.tensor_copy(out=kfi[:], in_=ku[:])
                k1024 = smallp.tile([128, NCH], F32, tag=f"k1k{ck}")
                nc.vector.tensor_scalar_mul(out=k1024[:], in0=kfi[:], scalar1=1024.0)
                npf = smallp.tile([128, NCH], F32, tag=f"npf{ck}")
                nc.vector.tensor_tensor(out=npf[:], in0=vf, in1=k1024[:],
                                        op=ALU.subtract)
                vidxf = smallp.tile([128, NCH], F32, tag=f"vif{ck}")
                nc.vector.tensor_tensor(out=vidxf[:], in0=npf[:], in1=crv_sb[:],
                                        op=ALU.add)
                vidx = smallp.tile([128, NCH], U16, tag=f"vi{ck}")
                nc.vector.tensor_copy(out=vidx[:], in_=vidxf[:])
                qidxf = smallp.tile([128, NCH], F32, tag=f"qif{ck}")
                nc.vector.tensor_tensor(out=qidxf[:], in0=kfi[:],
                                        in1=crq_sb[:], op=ALU.add)
                qidx = smallp.tile([128, NCH], U16, tag=f"qi{ck}")
                nc.vector.tensor_copy(out=qidx[:], in_=qidxf[:])

                ogv = smallp.tile([128, NCH * GRP], U32, tag=f"ogv{ck}")
                nc.gpsimd.indirect_copy(out=ogv[:], data=qrp_sb[ck][:],
                                        idxs=vidx[:],
                                        i_know_ap_gather_is_preferred=True)
                ogq = smallp.tile([128, NCH * GRP], F16, tag=f"ogq{ck}")
                nc.gpsimd.indirect_copy(out=ogq[:], data=qc16[ck][:],
                                        idxs=qidx[:],
                                        i_know_ap_gather_is_preferred=True)
                # diag pre-mask on the (otherwise idle) gpsimd queue
                d16b = d16_sb[:].rearrange("p (o m) -> p o m", o=1) \
                    .to_broadcast([128, NCH, GRP])
                qmul = smallp.tile([128, NCH * GRP], F16, tag=f"qm{ck}")
                nc.gpsimd.tensor_tensor(
                    out=qmul[:].rearrange("p (t m) -> p t m", m=GRP),
                    in0=ogq[:].rearrange("p (t m) -> p t m", m=GRP),
                    in1=d16b, op=ALU.mult)
                ogv_t[ck], ogq_t[ck] = ogv, qmul

            def chunk_reduce(ck):
                """Diagonal extraction (DVE) - emitted well after the gathers
                so the DVE queue head never waits on gpsimd."""
                lo, hi = ck * NCH, (ck + 1) * NCH
                ogv, qmul = ogv_t[ck], ogq_t[ck]
                d32b = d32_sb[:].rearrange("p (o m) -> p o m", o=1) \
                    .to_broadcast([128, NCH, GRP])
                vand = smallp.tile([128, NCH * GRP], U32, tag=f"va{ck}")
                nc.vector.tensor_tensor(
                    out=vand[:].rearrange("p (t m) -> p t m", m=GRP),
                    in0=ogv[:].rearrange("p (t m) -> p t m", m=GRP),
                    in1=d32b, op=ALU.bitwise_and)
                nc.vector.tensor_reduce(
                    out=vsel[:, lo:hi],
                    in_=vand[:].rearrange("p (t m) -> p t m", m=GRP),
                    axis=mybir.AxisListType.X, op=ALU.bitwise_or)
                nc.vector.tensor_reduce(
                    out=qvals[:, lo:hi],
                    in_=qmul[:].rearrange("p (t m) -> p t m", m=GRP),
                    axis=mybir.AxisListType.X, op=ALU.add)

            # ---- main loop: r outer so ref 0 overlaps refs 1/2 input DMA ----
            gather_at = {13: 0, 25: 1, 37: 2}
            reduce_at = {}
            for r in range(NREF):
                fhv = frh_sb[r][:].rearrange("c (y x) -> c y x", x=PW)
                flv = frl_sb[r][:].rearrange("c (y x) -> c y x", x=PW)
                for t in range(NT):
                    rt = r * NT + t
                    ps = psump.tile([128, 1024], F32, tag="ps")
                    lh = fth_sb[:, t * 128:(t + 1) * 128]
                    ll = ftl_sb[:, t * 128:(t + 1) * 128]
                    x0 = t * XB
                    rh1 = fhv[:, 0:HALF, x0:x0 + WX]
                    rh2 = fhv[:, HALF:WY, x0:x0 + WX]
                    rl1 = flv[:, 0:HALF, x0:x0 + WX]
                    rl2 = flv[:, HALF:WY, x0:x0 + WX]
                    nc.tensor.matmul(ps[:, 0:NHALF], lh, rh1,
                                     start=True, stop=False)
                    nc.tensor.matmul(ps[:, 512:512 + NHALF], lh, rh2,
                                     start=True, stop=False)
                    nc.tensor.matmul(ps[:, 0:NHALF], lh, rl1,
                                     start=False, stop=False)
                    nc.tensor.matmul(ps[:, 512:512 + NHALF], lh, rl2,
                                     start=False, stop=False)
                    nc.tensor.matmul(ps[:, 0:NHALF], ll, rh1,
                                     start=False, stop=True)
                    nc.tensor.matmul(ps[:, 512:512 + NHALF], ll, rh2,
                                     start=False, stop=True)

                    psv = ps[:].rearrange("p (b n) -> p b n", b=2)[:, :, 0:NHALF]
                    sc = streamp.tile([128, WIN], F16, tag="sc")
                    nc.vector._custom_dve(
                        amx,
                        out=sc[:].rearrange("p (b n) -> p b n", b=2),
                        in0=psv, in1=ttv,
                        accum_out=vfold[rt // NCH][:, rt % NCH:rt % NCH + 1],
                    )
                    qcv = qc16[rt // NCH][:, (rt % NCH) * WIN:(rt % NCH + 1) * WIN] \
                        .rearrange("p (b n) -> p b n", b=2)
                    nc.scalar.activation(
                        out=qcv, in_=psv,
                        func=mybir.ActivationFunctionType.Copy, scale=QSC)

                    if rt in gather_at:
                        chunk_gather(gather_at[rt])
                    if rt in reduce_at:
                        chunk_reduce(reduce_at[rt])
            chunk_gather(NCHK - 1)
            for _ck in range(NCHK):
                chunk_reduce(_ck)

            # ---- unpack gathered u8x4 values -> f32 ----
            valsf = smallp.tile([128, NRT * 4], F32, tag="valsf")
            nc.vector.tensor_copy(out=valsf[:], in_=vsel[:].bitcast(U8))

            # ---- softmax over refs + weighted sum ----
            qvv = qvals[:].rearrange("p (r t) -> p r t", r=NREF)
            m01 = smallp.tile([128, NT], F32, tag="m01")
            nc.vector.tensor_tensor(out=m01[:], in0=qvv[:, 0], in1=qvv[:, 1],
                                    op=ALU.max)
            mmx = smallp.tile([128, NT], F32, tag="mmx")
            nc.vector.tensor_tensor(out=mmx[:], in0=m01[:], in1=qvv[:, 2],
                                    op=ALU.max)
            es = []
            for r in range(NREF):
                e_ = smallp.tile([128, NT], F32, tag=f"e{r}")
                nc.vector.tensor_tensor(out=e_[:], in0=qvv[:, r], in1=mmx[:],
                                        op=ALU.subtract)
                nc.scalar.activation(out=e_[:], in_=e_[:],
                                     func=mybir.ActivationFunctionType.Exp,
                                     scale=QEXP)
                es.append(e_)
            ssum = smallp.tile([128, NT], F32, tag="ssum")
            nc.vector.tensor_tensor(out=ssum[:], in0=es[0][:], in1=es[1][:],
                                    op=ALU.add)
            nc.vector.tensor_tensor(out=ssum[:], in0=ssum[:], in1=es[2][:],
                                    op=ALU.add)
            rec = smallp.tile([128, NT], F32, tag="rec")
            nc.vector.reciprocal(out=rec[:], in_=ssum[:])
            nc.vector.tensor_scalar_mul(out=rec[:], in0=rec[:],
                                        scalar1=1.0 / 255.0)

            gvv = valsf[:].rearrange("p (r t c) -> p r t c", r=NREF, c=4)
            oacc = smallp.tile([128, NT * 4], F32, tag="oacc")
            oaccv = oacc[:].rearrange("p (t c) -> p t c", c=4)
            for r in range(NREF):
                w_ = smallp.tile([128, NT], F32, tag=f"w{r}")
                nc.vector.tensor_tensor(out=w_[:], in0=es[r][:], in1=rec[:],
                                        op=ALU.mult)
                wb = w_[:].rearrange("p (t o) -> p t o", o=1) \
                    .to_broadcast([128, NT, 4])
                if r == 0:
                    nc.vector.tensor_tensor(out=oaccv, in0=gvv[:, r], in1=wb,
                                            op=ALU.mult)
                else:
                    term = smallp.tile([128, NT * 4], F32, tag=f"term{r}")
                    termv = term[:].rearrange("p (t c) -> p t c", c=4)
                    nc.vector.tensor_tensor(out=termv, in0=gvv[:, r], in1=wb,
                                            op=ALU.mult)
                    nc.vector.tensor_tensor(out=oaccv, in0=oaccv, in1=termv,
                                            op=ALU.add)
            nc.sync.dma_start(out=out_d.ap(), in_=oacc[:])


    nc.compile()
    return nc


def _host_prep(feats_r, feats_t, quantized_r):
    """Build the 8 per-core input maps."""
    from numpy.lib.stride_tricks import sliding_window_view

    # prescaled, zero-padded refs; fp16 hi/lo split
    frp = np.zeros((NREF, C, H + 2 * RAD, PW), np.float32)
    frp[:, :, RAD:RAD + H, RAD:RAD + W] = feats_r[:, 0] * SCALE
    frh_full = frp.astype(np.float16)
    frl_full = (frp - frh_full.astype(np.float32)).astype(np.float16)

    ft = feats_t[0] * SCALE                      # [C, H, W] f32
    fth_full = ft.astype(np.float16)
    ftl_full = (ft - fth_full.astype(np.float32)).astype(np.float16)

    # t-table: valid -> 1024*k + n', invalid -> distinct decreasing negatives
    yl = np.arange(ROWS)[:, None, None, None]
    xl = np.arange(XB)[None, :, None, None]
    wy = np.arange(WY)[None, None, :, None]
    wx = np.arange(WX)[None, None, None, :]
    dy, dx = wy - yl, wx - xl
    valid = (dy >= 0) & (dy < PS) & (dx >= 0) & (dx < PS)
    kk = (wy * WX + wx).astype(np.float32)
    npr = (dy * PS + dx).astype(np.float32)
    tt = np.where(valid, 1024.0 * kk + npr,
                  (-1.0e30 * (1.0 + kk / 1024.0)).astype(np.float32))
    tt = np.ascontiguousarray(
        (tt + np.zeros((ROWS, XB, WY, WX), np.float32)).reshape(128, WIN),
        np.float32)

    # quantized refs: subsample, u8-encode, pad, per-pixel 13x13 patch tables
    qr = np.asarray(quantized_r[:, 0, :, ::SUB, ::SUB], np.float32)
    qr_u8 = np.clip(np.round(qr * 255.0), 0, 255).astype(np.uint8)
    qpad = np.zeros((NREF, CQ, H + 2 * RAD, W + 2 * RAD), np.uint8)
    qpad[:, :, RAD:RAD + H, RAD:RAD + W] = qr_u8
    sw = sliding_window_view(qpad, (PS, PS), axis=(2, 3))  # [r,c,H,W,13,13]

    crv = np.broadcast_to(
        (np.arange(NCH, dtype=np.float32) * NP)[None, :], (128, NCH))
    crv = np.ascontiguousarray(crv)
    crq = np.broadcast_to(
        (np.arange(NCH, dtype=np.float32) * WIN)[None, :], (128, NCH))
    crq = np.ascontiguousarray(crq)
    pm = np.arange(128) % GRP
    d32 = (pm[:, None] == np.arange(GRP)[None, :]).astype(np.uint32) * \
        np.uint32(0xFFFFFFFF)
    d16 = (pm[:, None] == np.arange(GRP)[None, :]).astype(np.float16)

    in_maps = []
    for k in range(NCORES):
        y0 = ROWS * k
        ft_h = np.ascontiguousarray(
            fth_full[:, y0:y0 + ROWS, :].reshape(C, ROWS, NT, XB)
            .transpose(0, 2, 1, 3).reshape(C, ROWS * W))
        ft_l = np.ascontiguousarray(
            ftl_full[:, y0:y0 + ROWS, :].reshape(C, ROWS, NT, XB)
            .transpose(0, 2, 1, 3).reshape(C, ROWS * W))
        fr_h = np.ascontiguousarray(
            frh_full[:, :, y0:y0 + WY, :].reshape(NREF, C, WY * PW))
        fr_l = np.ascontiguousarray(
            frl_full[:, :, y0:y0 + WY, :].reshape(NREF, C, WY * PW))

        # patch tables [p=(yl,xl), (r,t), n'=(dy,dx)] as u8x4 packed in u32
        blk = sw[:, :, y0:y0 + ROWS, :, :, :]          # [r, c, 16, 128, 13, 13]
        b2 = blk.reshape(NREF, CQ, ROWS, NT, XB, PS, PS)
        b3 = b2.transpose(2, 4, 0, 3, 5, 6, 1)         # [yl, xl, r, t, dy, dx, c]
        tbl = np.zeros((ROWS, XB, NREF, NT, NP, 4), np.uint8)
        tbl[..., :CQ] = np.ascontiguousarray(b3).reshape(
            ROWS, XB, NREF, NT, NP, CQ)
        qrp = tbl.reshape(128, NRT * NP * 4).view(np.uint32) \
            .reshape(128, NRT * NP)
        qrp = np.ascontiguousarray(
            qrp.reshape(128, NCHK, NCH * NP).transpose(1, 0, 2))

        in_maps.append({
            "fth": ft_h, "ftl": ft_l, "frh": fr_h, "frl": fr_l,
            "tt": tt, "qrp": qrp, "crv": crv, "crq": crq,
            "d32": d32, "d16": d16,
        })
    return in_maps


def _install_ntff_shim():
    """This container's antenv lacks axon_hooks, so run_bass_kernel_spmd's
    trace path can't find the NTFF profile hook. Inject the module and
    register the ctypes-based hook from the boot script. Best-effort."""
    try:
        import sys
        import types

        if "antenv.axon_hooks" in sys.modules:
            return
        mod = types.ModuleType("antenv.axon_hooks")
        holder = [None]
        mod.set_axon_ntff_profile_hook = lambda h: holder.__setitem__(0, h)
        mod.get_axon_ntff_profile_hook = lambda: holder[0]
        sys.modules["antenv.axon_hooks"] = mod
        import antenv

        antenv.axon_hooks = mod
        from trn_agent_boot.trn_boot import _ntff_profile_via_ctypes

        hook = _ntff_profile_via_ctypes("/opt/axon/libaxon_pjrt.so")
        if hook is not None:
            mod.set_axon_ntff_profile_hook(hook)
    except Exception as e:  # pragma: no cover - tracing is best-effort
        print(f"ntff shim install failed: {e}")


last_exec_time_ns = None
last_results = None


def kernel(feats_r, feats_t, quantized_r, ref_index=None, current_ind=None):
    global last_exec_time_ns, last_results
    feats_r = np.asarray(feats_r, np.float32)
    feats_t = np.asarray(feats_t, np.float32)
    quantized_r = np.asarray(quantized_r, np.float32)

    in_maps = _host_prep(feats_r, feats_t, quantized_r)

    if "nc" not in _CACHE:
        _CACHE["nc"] = _build_program()
    nc = _CACHE["nc"]

    trace = bool(int(os.environ.get("KERNEL_TRACE", "0")))
    kwargs = {}
    if trace:
        _install_ntff_shim()
        tdir = os.environ.get("KERNEL_TRACE_DIR")
        if tdir:
            os.makedirs(tdir, exist_ok=True)
            kwargs["tmpdir"] = tdir
    res = run_bass_kernel_spmd(
        nc, in_maps, list(range(NCORES)), trace=trace, **kwargs
    )
    last_exec_time_ns = res.exec_time_ns
    last_results = res.results

    out = np.concatenate(
        [_unshard_core(res.results[k]["out"]) for k in range(NCORES)], axis=1
    )
    return np.ascontiguousarray(out.reshape(1, CQ, H, W), np.float32)


def _unshard_core(raw):
    # raw [128, NT*4]: partition p=(yl,xl), free (t, c4) -> [CQ, ROWS, W]
    r = np.asarray(raw).reshape(ROWS, XB, NT, 4)[..., :CQ]
    return r.transpose(3, 0, 2, 1).reshape(CQ, ROWS, W)
